# revision 35
# baseline (speedup 1.0000x reference)
"""GroupMixAttention Trainium2 kernel (8-core SPMD, batch-parallel), v5.

Problem: x[16,256,32,32]; per group g (4 groups of 64 ch):
  Q/K/V = wq/wk/wv[g] @ xg   (xg = [64, 1024])
  scores = (Q^T K)/8 ; attn = softmax(scores, -1) ; out = V @ attn^T
then y = wo @ concat(out).

Sharding: data-parallel over batch, 2 batches per core, no collectives.

v5 = v3 two-group-interleaved steps (deepest independent PE work per
step, which is what actually hides LDWEIGHTS) plus:
  - PE warmup spin: ~4us of dependency-free tiny matmuls issued at
    t=0 so the HAM clock-gate reaches 8/8 (2.4 GHz) while the x DMAs
    are still in flight, instead of ramping mid-attention.
  - Q/K fold (U = blockdiag(wk^T wq) @ x), one dense K=128 matmul.
  - exp split ACT (native) / DVE (one-op Schraudolph fast-exp:
    int16(S*a+b) bits read as bf16 = exp(S/8), ~2% rms).
  - reciprocal_approx_fast + gpsimd partition_broadcast for softmax
    normalization; denominator rides col 0 of the PV weights so it
    lands on PSUM partition 0 (custom-DVE/gpsimd ops need base 0).
  - fp16 x/U/wu/wv/wo, bf16 E/VT.
"""

import os
import sys

import numpy as np

for _p in ("/opt/trn_rl_repo", "/root/.axon_site/_ro/trn_rl_repo"):
    if os.path.isdir(_p) and _p not in sys.path:
        sys.path.insert(0, _p)

import concourse.bass as bass
import concourse.mybir as mybir
import concourse.tile as tile
from concourse import bacc
from concourse.bass_utils import run_bass_kernel_spmd

F32 = mybir.dt.float32
F16 = mybir.dt.float16
BF16 = mybir.dt.bfloat16
I16 = mybir.dt.int16
EXP = mybir.ActivationFunctionType.Exp
MULT = mybir.AluOpType.mult
ADD = mybir.AluOpType.add
N_CORES = 8
B_PER_CORE = 2  # 16 batches / 8 cores
NT = 1024  # H*W
GD = 64    # group dim
ts = bass.ts

# Schraudolph fast-exp for exp(0.125*S) in bf16 bit space:
#   bits16 = int16(S * (0.125 * 2^7 / ln2) + (127 - C) * 2^7)
SCH_A = 0.125 * 128.0 / float(np.log(2.0))
SCH_B = (127.0 - 0.0575) * 128.0
DVE_MC = 6      # group-0 m-chunks < DVE_MC exp on DVE, rest on ACT
# (DVE is the slower exp engine; giving it the earlier in-step slot
# balances the two exp finish times against the S-bank reuse)
WARMUP_MMS = 110  # dependency-free spin matmuls covering the x-DMA window


def _build_program():
    nc = bacc.Bacc("TRN2", target_bir_lowering=False, debug=False,
                   num_devices=N_CORES)
    xs = nc.dram_tensor("xs", [B_PER_CORE, 2, 128, NT], F16,
                        kind="ExternalInput").ap()
    wu_bd = nc.dram_tensor("wu_bd", [2, 128, 128], F16,
                           kind="ExternalInput").ap()
    wv_bd = nc.dram_tensor("wv_bd", [2, 128, 128], F16,
                           kind="ExternalInput").ap()
    woT = nc.dram_tensor("woT", [2, 128, 256], F16, kind="ExternalInput").ap()
    y = nc.dram_tensor("y", [B_PER_CORE, 256, NT], F16,
                       kind="ExternalOutput").ap()

    BP = B_PER_CORE

    with tile.TileContext(nc) as tc:
        from contextlib import ExitStack
        with ExitStack() as ctx:
            const = ctx.enter_context(tc.tile_pool(name="const", bufs=1))
            up = ctx.enter_context(tc.tile_pool(name="up", bufs=1))
            ep = ctx.enter_context(tc.tile_pool(name="ep", bufs=4))
            op = ctx.enter_context(tc.tile_pool(name="op", bufs=2))
            sp = ctx.enter_context(tc.tile_pool(name="sp", bufs=2))
            psS = ctx.enter_context(
                tc.tile_pool(name="psS", bufs=1, space="PSUM"))
            psO = ctx.enter_context(
                tc.tile_pool(name="psO", bufs=1, space="PSUM"))

            # ---- PE warmup spin ------------------------------------------
            # Tiny dependency-free matmuls on a zeroed tile: keeps the PE
            # array active from t=0 so the HAM un-throttles to 2.4 GHz
            # during the input DMA wait. Output is never read.
            warm = const.tile([128, 64], F16, tag="warm", name="warm")
            nc.gpsimd.memset(warm[:], 0.0)
            psW = psS.tile([128, 64], F32, tag="S0", name="psW")
            for i in range(WARMUP_MMS):
                nc.tensor.matmul(psW[0:GD, :], warm[:], warm[:],
                                 start=True, stop=True)

            # ---- constants ----------------------------------------------
            wu_sb, wv_sb, wo_sb = [], [], []
            for p in range(2):
                t = const.tile([128, 128], F16, tag=f"wu{p}", name=f"wu{p}")
                nc.gpsimd.dma_start(t[:], wu_bd[p])
                wu_sb.append(t)
                t = const.tile([128, 128], F16, tag=f"wv{p}", name=f"wv{p}")
                nc.gpsimd.dma_start(t[:], wv_bd[p])
                wv_sb.append(t)
            for k in range(2):
                t = const.tile([128, 256], F16, tag=f"wo{k}", name=f"wo{k}")
                nc.gpsimd.dma_start(t[:], woT[k])
                wo_sb.append(t)

            # ---- x loads -------------------------------------------------
            xh = {}
            for b in range(BP):
                for p in range(2):
                    t = const.tile([128, NT], F16, tag=f"xh{b}{p}",
                                   name=f"xh{b}{p}")
                    nc.sync.dma_start(t[:], xs[b, p])
                    xh[b, p] = t

            # ---- prep: U projection + V^T for one (b, p) ----------------
            Uh = {}
            VT = {}

            def prep(b, p):
                x2 = xh[b, p]
                psU = psO.tile([128, NT], F32, tag="O0", name=f"psU{b}{p}")
                for nh in range(2):
                    s = ts(nh, 512)
                    nc.tensor.matmul(psU[:, s], wu_sb[p][:], x2[:, s],
                                     start=True, stop=True)
                u = up.tile([128, NT], F16, tag=f"Uh{b}{p}", name=f"Uh{b}{p}")
                nc.scalar.copy(u[:], psU[:])
                Uh[b, p] = u

                psV = psO.tile([128, 8, 128], F32, tag="O1", name=f"psV{b}{p}")
                for mc in range(8):
                    nc.tensor.matmul(
                        psV[:, mc, :], x2[:, ts(mc, 128)],
                        wv_sb[p][:], start=True, stop=True)
                # vt col 0 = ones -> denominator lands on psO partition 0
                # (reciprocal_approx_fast / partition_broadcast need base
                # partition 0); cols 64:128 = V^T -> psO rows 64:128.
                for g in range(2):
                    vt = up.tile([128, 8, 128], BF16, tag=f"VT{b}{p}{g}",
                                 name=f"VT{b}{p}{g}")
                    nc.gpsimd.memset(vt[:, :, 0:1], 1.0)
                    nc.gpsimd.memset(vt[:, :, 1:GD], 0.0)
                    nc.scalar.copy(vt[:, :, GD:128],
                                   psV[:, :, 64 * g:64 * g + GD])
                    VT[b, p, g] = vt

            # ---- attention loop for one (b, p) --------------------------
            PSO = {}

            def attn(b, p):
                x2 = xh[b, p]
                u = Uh[b, p]
                pso = [psO.tile([128, NT], F32, tag=f"O{g}",
                                name=f"psO{b}{p}{g}") for g in range(2)]
                PSO[b, p] = pso
                E = {}
                for step in range(9):
                    # Per step: S(g0), PV(g0, prev), S(g1), PV(g1, prev).
                    # Staggering the two exps like this leaves each one a
                    # full step of slack before its S bank is reused.
                    for g in range(2):
                        if step < 8:
                            mc = step
                            r = slice(64 * g, 64 * (g + 1))
                            S = psS.tile([128, NT], F32, tag=f"S{g}",
                                         name=f"S{b}{p}{g}_{mc}")
                            for nh in range(2):
                                s = ts(nh, 512)
                                nc.tensor.matmul(
                                    S[:, s], u[r, ts(mc, 128)],
                                    x2[r, s],
                                    start=True, stop=True,
                                    tile_position=(64 * g, 0))
                            e = ep.tile([128, NT], BF16, tag=f"E{g}",
                                        name=f"E{b}{p}{g}_{mc}")
                            if g == 0 and mc < DVE_MC:
                                nc.vector.tensor_scalar(
                                    out=e[:].bitcast(I16), in0=S[:],
                                    scalar1=SCH_A, scalar2=SCH_B,
                                    op0=MULT, op1=ADD)
                            else:
                                nc.scalar.activation(e[:], S[:], EXP,
                                                     scale=0.125)
                            E[g] = e
                        if step >= 1:
                            mc = step - 1
                            for nh in range(2):
                                s = ts(nh, 512)
                                nc.tensor.matmul(
                                    pso[g][:, s],
                                    VT[b, p, g][:, mc, :],
                                    E[g + 2][:, s],
                                    start=(mc == 0), stop=(mc == 7))
                        if g in E:
                            E[g + 2] = E.pop(g)

            # ---- normalize + evict for one (b, p) -----------------------
            outF = {}

            def norm(b, p):
                pso = PSO.pop((b, p))
                o = op.tile([128, NT], F16, tag=f"oF{p}", name=f"oF{b}{p}")
                outF[b, p] = o
                for g in range(2):
                    rec = sp.tile([1, NT], F32, tag="rec",
                                  name=f"rec{b}{p}{g}")
                    nc.vector.reciprocal_approx_fast(
                        rec[:], pso[g][0:1, :])
                    recB = sp.tile([GD, NT], F32, tag="recB",
                                   name=f"recB{b}{p}{g}")
                    nc.gpsimd.partition_broadcast(recB[:], rec[:])
                    nc.vector.tensor_tensor(
                        out=o[64 * g:64 * (g + 1), :],
                        in0=pso[g][GD:128, :],
                        in1=recB[:],
                        op=MULT)

            # ---- tail: out_proj + store ---------------------------------
            def tail(b):
                for ec in range(2):
                    psY = psO.tile([128, NT], F32, tag=f"O{ec}",
                                   name=f"psY{b}{ec}")
                    for nh in range(2):
                        s = ts(nh, 512)
                        for kc in range(2):
                            nc.tensor.matmul(
                                psY[:, s], wo_sb[kc][:, ts(ec, 128)],
                                outF[b, kc][:, s],
                                start=(kc == 0), stop=(kc == 1))
                    ysb = sp.tile([128, NT], F16, tag="ysb",
                                  name=f"ysb{b}{ec}")
                    nc.scalar.copy(ysb[:], psY[:])
                    nc.sync.dma_start(y[b][ts(ec, 128), :], ysb[:])

            # ---- schedule -----------------------------------------------
            prep(0, 0)
            prep(0, 1)
            attn(0, 0)
            norm(0, 0)
            prep(1, 0)
            attn(0, 1)
            norm(0, 1)
            prep(1, 1)
            attn(1, 0)
            norm(1, 0)
            tail(0)
            attn(1, 1)
            norm(1, 1)
            tail(1)

    nc.finalize()
    return nc


_NC_CACHE = None


def _get_nc():
    global _NC_CACHE
    if _NC_CACHE is None:
        _NC_CACHE = _build_program()
    return _NC_CACHE


def _prep_inputs(x, wq, wk, wv, wo):
    B = x.shape[0]
    xr = np.ascontiguousarray(x.reshape(B, 2, 128, NT), dtype=np.float16)
    wu = np.einsum('gdc,gde->gce', wk.astype(np.float64),
                   wq.astype(np.float64))
    wu_bd = np.zeros((2, 128, 128), dtype=np.float16)
    wv_bd = np.zeros((2, 128, 128), dtype=np.float16)
    for p in range(2):
        for g in range(2):
            sl = slice(64 * g, 64 * (g + 1))
            wu_bd[p, sl, sl] = wu[2 * p + g]
            wv_bd[p, sl, sl] = wv[2 * p + g].T
    woT = np.ascontiguousarray(wo.T.reshape(2, 128, 256), dtype=np.float16)
    return xr, wu_bd, wv_bd, woT


def run(x, wq, wk, wv, wo, trace=False, **trace_kwargs):
    x = np.asarray(x, dtype=np.float32)
    B, C, H, W = x.shape
    xr, wu_bd, wv_bd, woT = _prep_inputs(
        x, np.asarray(wq, np.float32), np.asarray(wk, np.float32),
        np.asarray(wv, np.float32), np.asarray(wo, np.float32))
    in_maps = []
    for c in range(N_CORES):
        in_maps.append({
            "xs": xr[c * B_PER_CORE:(c + 1) * B_PER_CORE],
            "wu_bd": wu_bd, "wv_bd": wv_bd, "woT": woT,
        })
    res = run_bass_kernel_spmd(_get_nc(), in_maps, list(range(N_CORES)),
                               trace=trace, **trace_kwargs)
    outs = [res.results[c]["y"] for c in range(N_CORES)]
    yfull = np.concatenate(outs, axis=0).reshape(B, C, H, W)
    return yfull.astype(np.float32), res


def kernel(x, wq, wk, wv, wo):
    out, _ = run(x, wq, wk, wv, wo, trace=False)
    return out


# revision 36
# speedup vs baseline: 1.2459x; 1.2459x over previous
"""GroupMixAttention Trainium2 kernel (8-core SPMD, batch-parallel), v5.

Problem: x[16,256,32,32]; per group g (4 groups of 64 ch):
  Q/K/V = wq/wk/wv[g] @ xg   (xg = [64, 1024])
  scores = (Q^T K)/8 ; attn = softmax(scores, -1) ; out = V @ attn^T
then y = wo @ concat(out).

Sharding: data-parallel over batch, 2 batches per core, no collectives.

v5 = v3 two-group-interleaved steps (deepest independent PE work per
step, which is what actually hides LDWEIGHTS) plus:
  - PE warmup spin: ~4us of dependency-free tiny matmuls issued at
    t=0 so the HAM clock-gate reaches 8/8 (2.4 GHz) while the x DMAs
    are still in flight, instead of ramping mid-attention.
  - Q/K fold (U = blockdiag(wk^T wq) @ x), one dense K=128 matmul.
  - exp split ACT (native) / DVE (one-op Schraudolph fast-exp:
    int16(S*a+b) bits read as bf16 = exp(S/8), ~2% rms).
  - reciprocal_approx_fast + gpsimd partition_broadcast for softmax
    normalization; denominator rides col 0 of the PV weights so it
    lands on PSUM partition 0 (custom-DVE/gpsimd ops need base 0).
  - fp16 x/U/wu/wv/wo, bf16 E/VT.
"""

import os
import sys

import numpy as np

for _p in ("/opt/trn_rl_repo", "/root/.axon_site/_ro/trn_rl_repo"):
    if os.path.isdir(_p) and _p not in sys.path:
        sys.path.insert(0, _p)

import concourse.bass as bass
import concourse.mybir as mybir
import concourse.tile as tile
from concourse import bacc
from concourse.bass_utils import run_bass_kernel_spmd

F32 = mybir.dt.float32
F16 = mybir.dt.float16
BF16 = mybir.dt.bfloat16
I16 = mybir.dt.int16
EXP = mybir.ActivationFunctionType.Exp
MULT = mybir.AluOpType.mult
ADD = mybir.AluOpType.add
N_CORES = 8
B_PER_CORE = 2  # 16 batches / 8 cores
NT = 1024  # H*W
GD = 64    # group dim
ts = bass.ts

# Schraudolph fast-exp for exp(0.125*S) in bf16 bit space:
#   bits16 = int16(S * (0.125 * 2^7 / ln2) + (127 - C) * 2^7)
SCH_A = 0.125 * 128.0 / float(np.log(2.0))
SCH_B = (127.0 - 0.0575) * 128.0
DVE_MC = 6      # group-1 m-chunks < DVE_MC exp on DVE, rest on ACT
WARMUP_MMS = 110  # dependency-free spin matmuls covering the x-DMA window


def _build_program():
    nc = bacc.Bacc("TRN2", target_bir_lowering=False, debug=False,
                   num_devices=N_CORES)
    xs = nc.dram_tensor("xs", [B_PER_CORE, 2, 128, NT], F16,
                        kind="ExternalInput").ap()
    wu_bd = nc.dram_tensor("wu_bd", [2, 128, 128], F16,
                           kind="ExternalInput").ap()
    wv_bd = nc.dram_tensor("wv_bd", [2, 128, 128], F16,
                           kind="ExternalInput").ap()
    woT = nc.dram_tensor("woT", [2, 128, 256], F16, kind="ExternalInput").ap()
    y = nc.dram_tensor("y", [B_PER_CORE, 256, NT], F16,
                       kind="ExternalOutput").ap()

    BP = B_PER_CORE

    with tile.TileContext(nc) as tc:
        from contextlib import ExitStack
        with ExitStack() as ctx:
            const = ctx.enter_context(tc.tile_pool(name="const", bufs=1))
            up = ctx.enter_context(tc.tile_pool(name="up", bufs=1))
            ep = ctx.enter_context(tc.tile_pool(name="ep", bufs=4))
            op = ctx.enter_context(tc.tile_pool(name="op", bufs=2))
            sp = ctx.enter_context(tc.tile_pool(name="sp", bufs=2))
            psS = ctx.enter_context(
                tc.tile_pool(name="psS", bufs=1, space="PSUM"))
            psO = ctx.enter_context(
                tc.tile_pool(name="psO", bufs=1, space="PSUM"))

            # ---- PE warmup spin ------------------------------------------
            # Tiny dependency-free matmuls on a zeroed tile: keeps the PE
            # array active from t=0 so the HAM un-throttles to 2.4 GHz
            # during the input DMA wait. Output is never read.
            warm = const.tile([128, 64], F16, tag="warm", name="warm")
            nc.gpsimd.memset(warm[:], 0.0)
            psW = psS.tile([128, 64], F32, tag="S0", name="psW")
            for i in range(WARMUP_MMS):
                nc.tensor.matmul(psW[0:GD, :], warm[:], warm[:],
                                 start=True, stop=True)

            # ---- constants ----------------------------------------------
            wu_sb, wv_sb, wo_sb = [], [], []
            for p in range(2):
                t = const.tile([128, 128], F16, tag=f"wu{p}", name=f"wu{p}")
                nc.gpsimd.dma_start(t[:], wu_bd[p])
                wu_sb.append(t)
                t = const.tile([128, 128], F16, tag=f"wv{p}", name=f"wv{p}")
                nc.gpsimd.dma_start(t[:], wv_bd[p])
                wv_sb.append(t)
            for k in range(2):
                t = const.tile([128, 256], F16, tag=f"wo{k}", name=f"wo{k}")
                nc.gpsimd.dma_start(t[:], woT[k])
                wo_sb.append(t)

            # ---- x loads -------------------------------------------------
            xh = {}
            for b in range(BP):
                for p in range(2):
                    t = const.tile([128, NT], F16, tag=f"xh{b}{p}",
                                   name=f"xh{b}{p}")
                    nc.sync.dma_start(t[:], xs[b, p])
                    xh[b, p] = t

            # ---- prep: U projection + V^T for one (b, p) ----------------
            Uh = {}
            VT = {}

            def prep(b, p):
                x2 = xh[b, p]
                psU = psO.tile([128, NT], F32, tag="O0", name=f"psU{b}{p}")
                for nh in range(2):
                    s = ts(nh, 512)
                    nc.tensor.matmul(psU[:, s], wu_sb[p][:], x2[:, s],
                                     start=True, stop=True)
                u = up.tile([128, NT], F16, tag=f"Uh{b}{p}", name=f"Uh{b}{p}")
                nc.scalar.copy(u[:], psU[:])
                Uh[b, p] = u

                psV = psO.tile([128, 8, 128], F32, tag="O1", name=f"psV{b}{p}")
                for mc in range(8):
                    nc.tensor.matmul(
                        psV[:, mc, :], x2[:, ts(mc, 128)],
                        wv_sb[p][:], start=True, stop=True)
                # vt col 0 = ones -> denominator lands on psO partition 0
                # (reciprocal_approx_fast / partition_broadcast need base
                # partition 0); cols 64:128 = V^T -> psO rows 64:128.
                for g in range(2):
                    vt = up.tile([128, 8, 128], BF16, tag=f"VT{b}{p}{g}",
                                 name=f"VT{b}{p}{g}")
                    nc.gpsimd.memset(vt[:, :, 0:1], 1.0)
                    nc.gpsimd.memset(vt[:, :, 1:GD], 0.0)
                    nc.scalar.copy(vt[:, :, GD:128],
                                   psV[:, :, 64 * g:64 * g + GD])
                    VT[b, p, g] = vt

            # ---- attention loop for one (b, p) --------------------------
            PSO = {}

            def attn(b, p):
                x2 = xh[b, p]
                u = Uh[b, p]
                pso = [psO.tile([128, NT], F32, tag=f"O{g}",
                                name=f"psO{b}{p}{g}") for g in range(2)]
                PSO[b, p] = pso
                E = {}
                for step in range(9):
                    # Per step: S(g0), PV(g0, prev), S(g1), PV(g1, prev).
                    # Staggering the two exps like this leaves each one a
                    # full step of slack before its S bank is reused.
                    for g in range(2):
                        if step < 8:
                            mc = step
                            r = slice(64 * g, 64 * (g + 1))
                            S = psS.tile([128, NT], F32, tag=f"S{g}",
                                         name=f"S{b}{p}{g}_{mc}")
                            for nh in range(2):
                                s = ts(nh, 512)
                                nc.tensor.matmul(
                                    S[:, s], u[r, ts(mc, 128)],
                                    x2[r, s],
                                    start=True, stop=True,
                                    tile_position=(64 * g, 0))
                            e = ep.tile([128, NT], BF16, tag=f"E{g}",
                                        name=f"E{b}{p}{g}_{mc}")
                            if g == 1 and mc < DVE_MC:
                                nc.vector.tensor_scalar(
                                    out=e[:].bitcast(I16), in0=S[:],
                                    scalar1=SCH_A, scalar2=SCH_B,
                                    op0=MULT, op1=ADD)
                            else:
                                nc.scalar.activation(e[:], S[:], EXP,
                                                     scale=0.125)
                            E[g] = e
                        if step >= 1:
                            mc = step - 1
                            for nh in range(2):
                                s = ts(nh, 512)
                                nc.tensor.matmul(
                                    pso[g][:, s],
                                    VT[b, p, g][:, mc, :],
                                    E[g + 2][:, s],
                                    start=(mc == 0), stop=(mc == 7))
                        if g in E:
                            E[g + 2] = E.pop(g)

            # ---- normalize + evict for one (b, p) -----------------------
            outF = {}

            def norm(b, p):
                pso = PSO.pop((b, p))
                o = op.tile([128, NT], F16, tag=f"oF{p}", name=f"oF{b}{p}")
                outF[b, p] = o
                for g in range(2):
                    rec = sp.tile([1, NT], F32, tag="rec",
                                  name=f"rec{b}{p}{g}")
                    nc.vector.reciprocal_approx_fast(
                        rec[:], pso[g][0:1, :])
                    recB = sp.tile([GD, NT], F32, tag="recB",
                                   name=f"recB{b}{p}{g}")
                    nc.gpsimd.partition_broadcast(recB[:], rec[:])
                    nc.vector.tensor_tensor(
                        out=o[64 * g:64 * (g + 1), :],
                        in0=pso[g][GD:128, :],
                        in1=recB[:],
                        op=MULT)

            # ---- tail: out_proj + store ---------------------------------
            def tail(b):
                for ec in range(2):
                    psY = psO.tile([128, NT], F32, tag=f"O{ec}",
                                   name=f"psY{b}{ec}")
                    for nh in range(2):
                        s = ts(nh, 512)
                        for kc in range(2):
                            nc.tensor.matmul(
                                psY[:, s], wo_sb[kc][:, ts(ec, 128)],
                                outF[b, kc][:, s],
                                start=(kc == 0), stop=(kc == 1))
                    ysb = sp.tile([128, NT], F16, tag="ysb",
                                  name=f"ysb{b}{ec}")
                    nc.scalar.copy(ysb[:], psY[:])
                    nc.sync.dma_start(y[b][ts(ec, 128), :], ysb[:])

            # ---- schedule -----------------------------------------------
            prep(0, 0)
            prep(0, 1)
            attn(0, 0)
            norm(0, 0)
            prep(1, 0)
            attn(0, 1)
            norm(0, 1)
            prep(1, 1)
            attn(1, 0)
            norm(1, 0)
            tail(0)
            attn(1, 1)
            norm(1, 1)
            tail(1)

    nc.finalize()
    return nc


_NC_CACHE = None


def _get_nc():
    global _NC_CACHE
    if _NC_CACHE is None:
        _NC_CACHE = _build_program()
    return _NC_CACHE


def _prep_inputs(x, wq, wk, wv, wo):
    B = x.shape[0]
    xr = np.ascontiguousarray(x.reshape(B, 2, 128, NT), dtype=np.float16)
    wu = np.einsum('gdc,gde->gce', wk.astype(np.float64),
                   wq.astype(np.float64))
    wu_bd = np.zeros((2, 128, 128), dtype=np.float16)
    wv_bd = np.zeros((2, 128, 128), dtype=np.float16)
    for p in range(2):
        for g in range(2):
            sl = slice(64 * g, 64 * (g + 1))
            wu_bd[p, sl, sl] = wu[2 * p + g]
            wv_bd[p, sl, sl] = wv[2 * p + g].T
    woT = np.ascontiguousarray(wo.T.reshape(2, 128, 256), dtype=np.float16)
    return xr, wu_bd, wv_bd, woT


def run(x, wq, wk, wv, wo, trace=False, **trace_kwargs):
    x = np.asarray(x, dtype=np.float32)
    B, C, H, W = x.shape
    xr, wu_bd, wv_bd, woT = _prep_inputs(
        x, np.asarray(wq, np.float32), np.asarray(wk, np.float32),
        np.asarray(wv, np.float32), np.asarray(wo, np.float32))
    in_maps = []
    for c in range(N_CORES):
        in_maps.append({
            "xs": xr[c * B_PER_CORE:(c + 1) * B_PER_CORE],
            "wu_bd": wu_bd, "wv_bd": wv_bd, "woT": woT,
        })
    res = run_bass_kernel_spmd(_get_nc(), in_maps, list(range(N_CORES)),
                               trace=trace, **trace_kwargs)
    outs = [res.results[c]["y"] for c in range(N_CORES)]
    yfull = np.concatenate(outs, axis=0).reshape(B, C, H, W)
    return yfull.astype(np.float32), res


def kernel(x, wq, wk, wv, wo):
    out, _ = run(x, wq, wk, wv, wo, trace=False)
    return out
